# revision 10
# baseline (speedup 1.0000x reference)
"""Trainium2 Bass kernel for nn_MultiHeadAttention_77455440216274.

Reference computation (B=2, S=2048, D=1024, H=16, depth=64):
    q = split_heads(q @ wq); k = split_heads(k @ wk); v = split_heads(v @ wv)
    scores = einsum('bhqd,bhkd->bhqk', q, k) / 8
    scores = where(causal_mask, scores, -8.0)
    attn = softmax(scores - rowmax)
    out = merge_heads(einsum('bhqk,bhkd->bhqd', attn, v))

Sharding: 8 cores = 2 batches x 4 head-groups. Core i handles batch i//4
and heads [4*(i%4), 4*(i%4)+4) (columns 256*(i%4) .. of wq/wk/wv).

Per-core algorithm (S=2048, 4 heads of depth 64, all fp32 math):
  - host pre-transposes q/k/v[b] to [D, S] so projections need no on-chip
    input transpose.
  - projections: qT/kT = w.T @ x.T in [d, S] layout (heads packed 2 per
    128-partition tile); v in natural [S, d] layout. qT scaled by 1/8
    (exact power of 2).
  - per (i-tile, head): bf16 stats matmul pass -> approximate row max m
    (exact when the true causal max <= -8; otherwise any m close to max
    cancels in softmax). Masked positions forced to exactly -8.
  - fp32 scores in 512-col chunks -> causal mask fix on diagonal block ->
    exp(s - m) on ACT with per-chunk row-sum accumulation -> PE-transpose
    of p blocks for the PV matmul.
  - PV: out.T[d, i] accumulated in PSUM with v as stationary operand,
    two heads col-packed in one 128-partition PSUM tile. The fully-masked
    upper region contributes exp(-8 - m_i) * suffix_sum(v) via rank-1
    matmuls. Result back-transposed, normalized by 1/Z, DMA'd out.
"""

import contextlib
import sys

sys.path.insert(0, "/opt/trn_rl_repo")

import numpy as np

import concourse.bass as bass
import concourse.tile as tile
from concourse import bacc, mybir
from concourse.bass_utils import run_bass_kernel_spmd
from concourse.masks import make_identity, make_upper_triangular

F32 = mybir.dt.float32
F32R = mybir.dt.float32r
BF16 = mybir.dt.bfloat16

import os
_KCFG = os.environ.get("KCFG", "vproj_r,pv_r,pc_r")
CFG_PROJ_R = "proj_r" in _KCFG      # q/k projections in f32r (imprecise!)
CFG_VPROJ_R = "vproj_r" in _KCFG    # v projection in f32r (safe)
CFG_SCORE_R = "score_r" in _KCFG    # scores in f32r (imprecise!)
CFG_PV_R = "pv_r" in _KCFG          # p@v matmul in f32r (safe)
CFG_PC_R = "pc_r" in _KCFG          # exp output + p transposes in f32r
PROJ_DT = F32R if CFG_PROJ_R else F32
VPROJ_DT = F32R if (CFG_PROJ_R or CFG_VPROJ_R) else F32
SCORE_DT = F32R if CFG_SCORE_R else F32
PV_DT = F32R if CFG_PV_R else F32
PC_DT = F32R if (CFG_PC_R and CFG_PV_R) else F32

P = 128          # partitions
S = 2048         # sequence length
D = 1024         # model dim
NT = S // P      # 16 i/j tiles
KT = D // P      # 8 contraction tiles for projections
HC = 256         # head columns per core (4 heads x 64)
DEP = 64         # head depth
CH = 512         # chunk size (PSUM bank = 512 fp32)
NICH = S // CH   # 4 i-chunks


def _emit(nc, tc):
    xqT = nc.dram_tensor("xqT", [D, S], PROJ_DT, kind="ExternalInput").ap()
    xkT = nc.dram_tensor("xkT", [D, S], PROJ_DT, kind="ExternalInput").ap()
    xvT = nc.dram_tensor("xvT", [D, S], VPROJ_DT, kind="ExternalInput").ap()
    wqd = nc.dram_tensor("wq", [D, HC], PROJ_DT, kind="ExternalInput").ap()
    wkd = nc.dram_tensor("wk", [D, HC], PROJ_DT, kind="ExternalInput").ap()
    wvd = nc.dram_tensor("wv", [D, HC], VPROJ_DT, kind="ExternalInput").ap()
    outd = nc.dram_tensor("out", [S, HC], F32, kind="ExternalOutput").ap()

    ctx = contextlib.ExitStack()
    with ctx:
        cons = ctx.enter_context(tc.tile_pool(name="cons", bufs=1))
        ident = cons.tile([P, P], F32)
        make_identity(nc, ident)
        ident_r = cons.tile([P, P], PC_DT)
        nc.vector.tensor_copy(out=ident_r[:], in_=ident[:])
        # 1 where strictly upper-triangular (col > row) = masked-out region
        umask = cons.tile([P, P], mybir.dt.int8)
        make_upper_triangular(nc, umask, val=1.0, diag=False)
        neg8 = cons.tile([P, P], F32)
        nc.vector.memset(neg8, -8.0)
        ones_col = cons.tile([P, 1], F32)
        nc.vector.memset(ones_col, 1.0)
        ones_r = cons.tile([P, 1], PV_DT)
        nc.vector.tensor_copy(out=ones_r[:], in_=ones_col[:])
        neg8_col = cons.tile([P, 1], F32)
        nc.vector.memset(neg8_col, -8.0)

        # persistent SBUF: projected tensors
        persist = ctx.enter_context(tc.tile_pool(name="persist", bufs=1))
        qT = [persist.tile([P, S], SCORE_DT, tag=f"qT{d}", name=f"qT{d}") for d in range(2)]
        kTt = [persist.tile([P, S], SCORE_DT, tag=f"kT{d}", name=f"kT{d}") for d in range(2)]
        qB = [persist.tile([P, S], BF16, tag=f"qB{d}", name=f"qB{d}") for d in range(2)]
        kB = [persist.tile([P, S], BF16, tag=f"kB{d}", name=f"kB{d}") for d in range(2)]
        vS = [persist.tile([P, HC], PV_DT, tag=f"v{j}", name=f"v{j}") for j in range(NT)]
        # column sums / suffix sums of v rows, packed on partition 0:
        # cs[0, 256*t : 256*t+256] = sum of v rows in j-tile t
        cs_flat = persist.tile([1, NT * HC], F32, tag="cs")
        sufv = persist.tile([1, NT * HC], F32, tag="sufv")

        # ---------------- phase 1: projections ----------------
        with tc.tile_pool(name="wpool", bufs=1) as wpool, \
             tc.tile_pool(name="xin", bufs=1) as xin, \
             tc.tile_pool(name="pj", bufs=4, space="PSUM") as pj, \
             tc.tile_pool(name="pjv", bufs=2, space="PSUM") as pjv, \
             tc.tile_pool(name="pjcs", bufs=2, space="PSUM") as pjcs:

            wtiles = {}
            for wname, wdram in (("wq", wqd), ("wk", wkd), ("wv", wvd)):
                tl = []
                wdt = VPROJ_DT if wname == "wv" else PROJ_DT
                for kt in range(KT):
                    w_t = wpool.tile([P, HC], wdt, tag=f"{wname}{kt}", name=f"w{wname}{kt}")
                    nc.sync.dma_start(out=w_t[:], in_=wdram[kt * P:(kt + 1) * P, :])
                    tl.append(w_t)
                wtiles[wname] = tl

            # q and k projections -> transposed layout [256, S] as 2 tiles
            for xdram, wname, dstF, dstB, scl in (
                (xqT, "wq", qT, qB, 0.125),
                (xkT, "wk", kTt, kB, 1.0),
            ):
                xt = []
                for kt in range(KT):
                    x_t = xin.tile([P, S], PROJ_DT, tag=f"x{kt}", name=f"xt{kt}")
                    nc.sync.dma_start(out=x_t[:], in_=xdram[kt * P:(kt + 1) * P, :])
                    xt.append(x_t)
                for d2 in range(2):
                    for c in range(S // CH):
                        ps = pj.tile([P, CH], F32, tag="pj", name="pjt")
                        for kt in range(KT):
                            nc.tensor.matmul(
                                ps[:],
                                wtiles[wname][kt][:, d2 * P:(d2 + 1) * P],
                                xt[kt][:, c * CH:(c + 1) * CH],
                                start=(kt == 0), stop=(kt == KT - 1),
                            )
                        nc.vector.tensor_scalar_mul(
                            dstF[d2][:, c * CH:(c + 1) * CH], ps[:], scl)
                        nc.scalar.mul(dstB[d2][:, c * CH:(c + 1) * CH], ps[:], scl)

            # v projection -> natural layout [S, 256] as 16 tiles + colsums
            xt = []
            for kt in range(KT):
                x_t = xin.tile([P, S], VPROJ_DT, tag=f"x{kt}", name=f"xvt{kt}")
                nc.sync.dma_start(out=x_t[:], in_=xvT[kt * P:(kt + 1) * P, :])
                xt.append(x_t)
            for jt in range(NT):
                ps = pjv.tile([P, HC], F32, tag="pjv", name="pjvt")
                for kt in range(KT):
                    nc.tensor.matmul(
                        ps[:],
                        xt[kt][:, jt * P:(jt + 1) * P],
                        wtiles["wv"][kt][:],
                        start=(kt == 0), stop=(kt == KT - 1),
                    )
                nc.vector.tensor_copy(out=vS[jt][:], in_=ps[:])
                psc = pjcs.tile([1, HC], F32, tag="pjcs", name="pjcst")
                nc.tensor.matmul(psc[:], ones_r[:], vS[jt][:], start=True, stop=True)
                nc.scalar.copy(out=cs_flat[0:1, jt * HC:(jt + 1) * HC], in_=psc[:])

        # suffix sums: sufv[t] = sum of colsums of j-tiles > t
        nc.vector.memset(sufv[0:1, (NT - 1) * HC: NT * HC], 0.0)
        for t in range(NT - 2, -1, -1):
            nc.vector.tensor_add(
                sufv[0:1, t * HC:(t + 1) * HC],
                sufv[0:1, (t + 1) * HC:(t + 2) * HC],
                cs_flat[0:1, (t + 1) * HC:(t + 2) * HC],
            )

        # ---------------- phase 2: attention ----------------
        with tc.tile_pool(name="stp", bufs=2, space="PSUM") as stp, \
             tc.tile_pool(name="scp0", bufs=1, space="PSUM") as scp0, \
             tc.tile_pool(name="scp1", bufs=1, space="PSUM") as scp1, \
             tc.tile_pool(name="trp", bufs=2, space="PSUM") as trp, \
             tc.tile_pool(name="otp", bufs=1, space="PSUM") as otp, \
             tc.tile_pool(name="pch", bufs=6) as pchp, \
             tc.tile_pool(name="ptrow", bufs=1) as ptp, \
             tc.tile_pool(name="small", bufs=2) as smp, \
             tc.tile_pool(name="hold", bufs=1) as hold, \
             tc.tile_pool(name="osb", bufs=1) as osbp:

            for ic in range(NICH):
                tlist = list(range(4 * ic, 4 * ic + 4))
                outsb = {}
                for t in tlist:
                    o_t = osbp.tile([P, HC], F32, tag=f"osb{t % 4}", name=f"osbt{t%4}")
                    outsb[t] = o_t
                for pair in range(2):
                    heads = (2 * pair, 2 * pair + 1)
                    ptrow = [
                        [ptp.tile([P, CH], PV_DT, tag=f"pt{hh}_{jb}", name=f"pt{hh}_{jb}")
                         for jb in range(4 * ic + 4)]
                        for hh in range(2)
                    ]
                    recs = {}
                    e8rows = {}
                    for t in tlist:
                        L = P * (t + 1)
                        nch = (L + CH - 1) // CH
                        dlo = t * P - (nch - 1) * CH  # diag offset in last chunk
                        scps = (scp0, scp1)
                        # --- bf16 stats pass: row maxes ---
                        rmp = [smp.tile([P, nch], F32, tag=f"rmp{hh}", name=f"rmp{hh}")
                               for hh in range(2)]
                        for c in range(nch):
                            cl = min(CH, L - c * CH)
                            for hh in range(2):
                                pslo, pshi = hh * DEP, (hh + 1) * DEP
                                st = stp.tile([P, CH], F32, tag="st", name="stt")
                                nc.tensor.matmul(
                                    st[:, :cl],
                                    qB[pair][pslo:pshi, t * P:(t + 1) * P],
                                    kB[pair][pslo:pshi, c * CH:c * CH + cl],
                                    start=True, stop=True,
                                )
                                if c == nch - 1:
                                    nc.vector.copy_predicated(
                                        st[:, dlo:dlo + P], umask[:], neg8[:])
                                nc.vector.reduce_max(
                                    rmp[hh][:, c:c + 1], st[:, :cl],
                                    axis=mybir.AxisListType.X)
                        negm = []
                        e8m = []
                        for hh in range(2):
                            nm = smp.tile([P, 1], F32, tag=f"negm{hh}", name=f"negm{hh}")
                            nc.vector.tensor_reduce(
                                nm[:], rmp[hh][:], axis=mybir.AxisListType.X,
                                op=mybir.AluOpType.max, negate=True)
                            nc.vector.tensor_scalar_min(nm[:], nm[:], 8.0)
                            negm.append(nm)
                            e8 = smp.tile([P, 1], F32, tag=f"e8m{hh}", name=f"e8m{hh}")
                            nc.scalar.activation(
                                e8[:], nm[:], mybir.ActivationFunctionType.Exp,
                                bias=neg8_col[:])
                            e8m.append(e8)
                            pse = trp.tile([1, P], F32, tag="tr", name="tre")
                            nc.tensor.transpose(pse[:], e8[:], ident[:])
                            e8r = hold.tile([1, P], F32, tag=f"e8r{hh}_{t % 4}", name=f"e8r{hh}_{t%4}")
                            nc.scalar.copy(out=e8r[:], in_=pse[:])
                            e8rows[(hh, t)] = e8r
                        # --- fp32 scores + exp + transpose ---
                        zp = [smp.tile([P, nch], F32, tag=f"zp{hh}", name=f"zp{hh}")
                              for hh in range(2)]
                        for c in range(nch):
                            cl = min(CH, L - c * CH)
                            for hh in range(2):
                                pslo, pshi = hh * DEP, (hh + 1) * DEP
                                sc = scps[hh].tile([P, CH], F32, tag=f"sc{hh}", name=f"sct{hh}")
                                nc.tensor.matmul(
                                    sc[:, :cl],
                                    qT[pair][pslo:pshi, t * P:(t + 1) * P],
                                    kTt[pair][pslo:pshi, c * CH:c * CH + cl],
                                    start=True, stop=True,
                                )
                                if c == nch - 1:
                                    nc.vector.copy_predicated(
                                        sc[:, dlo:dlo + P], umask[:], neg8[:])
                                pc = pchp.tile([P, CH], PC_DT, tag="pch", name="pcht")
                                nc.scalar.activation(
                                    pc[:, :cl], sc[:, :cl],
                                    mybir.ActivationFunctionType.Exp,
                                    bias=negm[hh][:],
                                    accum_out=zp[hh][:, c:c + 1])
                                for b in range(cl // P):
                                    jb = c * (CH // P) + b
                                    ptr = trp.tile([P, P], PC_DT, tag="tr", name="trt")
                                    nc.tensor.transpose(
                                        ptr[:], pc[:, b * P:(b + 1) * P], ident_r[:])
                                    nc.vector.tensor_copy(
                                        out=ptrow[hh][jb][:, (t - 4 * ic) * P:
                                                          (t - 4 * ic + 1) * P],
                                        in_=ptr[:])
                        # --- Z and 1/Z ---
                        cnt = float(S - L)
                        for hh in range(2):
                            zc = smp.tile([P, 1], F32, tag=f"zc{hh}", name=f"zc{hh}")
                            nc.vector.reduce_sum(
                                zc[:], zp[hh][:], axis=mybir.AxisListType.X)
                            zm = smp.tile([P, 1], F32, tag=f"zm{hh}", name=f"zm{hh}")
                            nc.vector.tensor_scalar_mul(zm[:], e8m[hh][:], cnt)
                            nc.vector.tensor_add(zc[:], zc[:], zm[:])
                            rc = hold.tile([P, 1], F32, tag=f"rec{hh}_{t % 4}", name=f"rec{hh}_{t%4}")
                            nc.vector.reciprocal(rc[:], zc[:])
                            recs[(hh, t)] = rc
                    # --- PV: outT[d, i] accumulation, one psum tile per head ---
                    last_jb = 4 * ic + 3
                    oTs = []
                    for hh in range(2):
                        o_h = otp.tile([DEP, CH], F32, tag=f"oT{hh}", name=f"oTt{hh}")
                        oTs.append(o_h)
                        vcols = pair * P + hh * DEP
                        for jb in range(last_jb + 1):
                            i0 = max(0, jb - 4 * ic) * P
                            nc.tensor.matmul(
                                o_h[:, i0:CH],
                                vS[jb][:, vcols:vcols + DEP],
                                ptrow[hh][jb][:, i0:CH],
                                start=(jb == 0), stop=False,
                            )
                        for t in tlist:
                            io = (t - 4 * ic) * P
                            scols = t * HC + pair * P + hh * DEP
                            nc.tensor.matmul(
                                o_h[:, io:io + P],
                                sufv[0:1, scols:scols + DEP],
                                e8rows[(hh, t)][:],
                                start=False, stop=(t == tlist[-1]),
                            )
                    # --- back-transpose + normalize ---
                    for hh in range(2):
                        oTsb = pchp.tile([DEP, CH], F32, tag="oTsb", name="oTsbt")
                        nc.any.tensor_copy(out=oTsb[:], in_=oTs[hh][:])
                        for t in tlist:
                            io = (t - 4 * ic) * P
                            ptr2 = trp.tile([P, P], F32, tag="tr", name="trt")
                            nc.tensor.transpose(
                                ptr2[:, 0:DEP], oTsb[:, io:io + P],
                                ident[0:DEP, 0:DEP])
                            oc = pair * P + hh * DEP
                            nc.vector.tensor_scalar_mul(
                                outsb[t][:, oc:oc + DEP],
                                ptr2[:, 0:DEP],
                                recs[(hh, t)][:])
                for t in tlist:
                    nc.sync.dma_start(
                        out=outd[t * P:(t + 1) * P, :], in_=outsb[t][:])


_PROGRAM_CACHE = {}


def _build():
    if "nc" in _PROGRAM_CACHE:
        return _PROGRAM_CACHE["nc"]
    nc = bacc.Bacc(trn_type="TRN2", target_bir_lowering=False, debug=False)
    with tile.TileContext(nc) as tc:
        _emit(nc, tc)
    nc.compile()
    _PROGRAM_CACHE["nc"] = nc
    return nc


def kernel(q, k, v, mask, wq, wk, wv, _collect_results=None):
    q = np.asarray(q, dtype=np.float32)
    k = np.asarray(k, dtype=np.float32)
    v = np.asarray(v, dtype=np.float32)
    wq = np.asarray(wq, dtype=np.float32)
    wk = np.asarray(wk, dtype=np.float32)
    wv = np.asarray(wv, dtype=np.float32)
    mask = np.asarray(mask)

    # the device program hardcodes the causal structure
    mrow = np.broadcast_to(mask, (1, 1, S, S)).reshape(S, S)
    assert mrow.dtype == np.bool_ or mrow.dtype == bool or True
    tril = np.tril(np.ones((S, S), dtype=bool))
    if not np.array_equal(mrow.astype(bool), tril):
        raise NotImplementedError("kernel compiled for causal (tril) mask only")

    nc = _build()

    in_maps = []
    for core in range(8):
        b, m = divmod(core, 4)
        cols = slice(HC * m, HC * (m + 1))
        in_maps.append({
            "xqT": np.ascontiguousarray(q[b].T),
            "xkT": np.ascontiguousarray(k[b].T),
            "xvT": np.ascontiguousarray(v[b].T),
            "wq": np.ascontiguousarray(wq[:, cols]),
            "wk": np.ascontiguousarray(wk[:, cols]),
            "wv": np.ascontiguousarray(wv[:, cols]),
        })

    trace = _collect_results is not None
    res = run_bass_kernel_spmd(nc, in_maps, core_ids=list(range(8)), trace=trace)
    if _collect_results is not None:
        _collect_results.append(res)

    out = np.empty((2, S, D), dtype=np.float32)
    for core in range(8):
        b, m = divmod(core, 4)
        out[b, :, HC * m: HC * (m + 1)] = res.results[core]["out"]
    return out


# revision 20
# speedup vs baseline: 1.1264x; 1.1264x over previous
"""Trainium2 Bass kernel for nn_MultiHeadAttention_77455440216274.

Reference computation (B=2, S=2048, D=1024, H=16, depth=64):
    q = split_heads(q @ wq); k = split_heads(k @ wk); v = split_heads(v @ wv)
    scores = einsum('bhqd,bhkd->bhqk', q, k) / 8
    scores = where(causal_mask, scores, -8.0)
    attn = softmax(scores - rowmax)
    out = merge_heads(einsum('bhqk,bhkd->bhqd', attn, v))

Sharding: 8 cores = 2 batches x 4 head-groups. Core i handles batch i//4
and heads [4*(i%4), 4*(i%4)+4) (columns 256*(i%4) .. of wq/wk/wv).

Per-core algorithm ("fold" variant; S=2048, 4 heads of depth 64):
  - host pre-transposes q/k/v[b] to [D, S]; projections in fp32. q scaled
    by 1/8 (exact). Per head, q~ = [qT_h ; m~-row] and k~ = [kT_h ; -1-row]
    are kept as [65, S] tiles so the scores matmul computes s - m~ directly.
  - bf16 stats pass gives the approximate row max m~ (exact -8 clamp when
    the true causal max <= -8; otherwise m~ cancels in softmax).
  - scores are computed TRANSPOSED [j, i] in fp32, exp'd by ACT straight
    into f32r p-tiles (no transposes needed for the PV matmul). Masked
    diagonal-block entries are fixed to exp(-8 - m~_i) via a gpsimd
    partition-broadcast + predicated copy.
  - PV: out.T[d, i] = v~.T @ p.T with v~ = [v | 1] in f32r, so PSUM row 64
    accumulates the softmax denominator Z for free. The fully-masked upper
    region contributes rank-1 updates [sufv_t | cnt_t] x e8m-row. A final
    PE back-transpose yields [i, d] plus the Z column; normalize by 1/Z.
"""

import contextlib
import os
import sys

sys.path.insert(0, "/opt/trn_rl_repo")

import numpy as np

import concourse.bass as bass
import concourse.tile as tile
from concourse import bacc, mybir
from concourse.bass_utils import run_bass_kernel_spmd
from concourse.masks import make_identity, make_lower_triangular, \
    make_upper_triangular

F32 = mybir.dt.float32
F32R = mybir.dt.float32r
BF16 = mybir.dt.bfloat16

P = 128          # partitions
S = 2048         # sequence length
D = 1024         # model dim
NT = S // P      # 16 i/j tiles
KT = D // P      # 8 contraction tiles for projections
HC = 256         # head columns per core (4 heads x 64)
DEP = 64         # head depth
CH = 512         # chunk size (PSUM bank = 512 fp32)
NICH = S // CH   # 4 i-chunks
VW = DEP + 1     # 65: v columns + ones column
SB = 4 * VW      # 260: sufv block stride per i-tile


def _emit(nc, tc):
    xqT = nc.dram_tensor("xqT", [D, S], F32, kind="ExternalInput").ap()
    xkT = nc.dram_tensor("xkT", [D, S], F32, kind="ExternalInput").ap()
    xvT = nc.dram_tensor("xvT", [D, S], F32, kind="ExternalInput").ap()
    wqd = nc.dram_tensor("wq", [D, HC], F32, kind="ExternalInput").ap()
    wkd = nc.dram_tensor("wk", [D, HC], F32, kind="ExternalInput").ap()
    wvd = nc.dram_tensor("wv", [D, HC], F32, kind="ExternalInput").ap()
    outd = nc.dram_tensor("out", [S, HC], F32, kind="ExternalOutput").ap()

    ctx = contextlib.ExitStack()
    with ctx:
        cons = ctx.enter_context(tc.tile_pool(name="cons", bufs=1))
        ident = cons.tile([P, P], F32)
        make_identity(nc, ident)
        ident_r = cons.tile([P, P], F32R)
        nc.vector.tensor_copy(out=ident_r[:], in_=ident[:])
        # int8 masks: 1 where the position is masked out
        umask = cons.tile([P, P], mybir.dt.int8)   # col > row ([i,j] layout)
        make_upper_triangular(nc, umask, val=1.0, diag=False)
        lmask = cons.tile([P, P], mybir.dt.int8)   # row > col ([j,i] layout)
        make_lower_triangular(nc, lmask, val=1.0, diag=False)
        neg8 = cons.tile([P, P], F32)
        nc.vector.memset(neg8, -8.0)
        ones_col = cons.tile([P, 1], F32)
        nc.vector.memset(ones_col, 1.0)
        ones_r = cons.tile([P, 1], F32R)
        nc.vector.tensor_copy(out=ones_r[:], in_=ones_col[:])
        neg8_col = cons.tile([P, 1], F32)
        nc.vector.memset(neg8_col, -8.0)

        # persistent SBUF
        persist = ctx.enter_context(tc.tile_pool(name="persist", bufs=1))
        q65 = [persist.tile([VW, S], F32, tag=f"q65_{h}", name=f"q65_{h}")
               for h in range(4)]
        k65 = [persist.tile([VW, S], F32, tag=f"k65_{h}", name=f"k65_{h}")
               for h in range(4)]
        qB = [persist.tile([P, S], BF16, tag=f"qB{d}", name=f"qB{d}")
              for d in range(2)]
        kB = [persist.tile([P, S], BF16, tag=f"kB{d}", name=f"kB{d}")
              for d in range(2)]
        # v with a ones column per head: [v_h (64) | 1] x 4 heads
        vS = [persist.tile([P, SB], F32R, tag=f"v{j}", name=f"v{j}")
              for j in range(NT)]
        cs_flat = persist.tile([1, NT * SB], F32, tag="cs")
        sufv = persist.tile([1, NT * SB], F32, tag="sufv")

        for h in range(4):
            nc.vector.memset(k65[h][DEP:VW, :], -1.0)

        # ---------------- phase 1: projections ----------------
        with tc.tile_pool(name="wpool", bufs=1) as wpool, \
             tc.tile_pool(name="xin", bufs=1) as xin, \
             tc.tile_pool(name="pj", bufs=4, space="PSUM") as pj, \
             tc.tile_pool(name="pjv", bufs=2, space="PSUM") as pjv, \
             tc.tile_pool(name="pjcs", bufs=2, space="PSUM") as pjcs:

            wtiles = {}
            for wname, wdram in (("wq", wqd), ("wk", wkd), ("wv", wvd)):
                tl = []
                for kt in range(KT):
                    w_t = wpool.tile([P, HC], F32, tag=f"{wname}{kt}",
                                     name=f"w{wname}{kt}")
                    nc.sync.dma_start(out=w_t[:], in_=wdram[kt * P:(kt + 1) * P, :])
                    tl.append(w_t)
                wtiles[wname] = tl

            # q and k projections -> [65, S] per-head tiles (rows 0:64)
            for xdram, wname, dst65, dstB, scl in (
                (xqT, "wq", q65, qB, 0.125),
                (xkT, "wk", k65, kB, 1.0),
            ):
                for c in range(S // CH):
                    xt = []
                    for kt in range(KT):
                        x_t = xin.tile([P, CH], F32, tag=f"x{kt}", name=f"xt{kt}")
                        nc.sync.dma_start(
                            out=x_t[:],
                            in_=xdram[kt * P:(kt + 1) * P, c * CH:(c + 1) * CH])
                        xt.append(x_t)
                    for d2 in range(2):
                        ps = pj.tile([P, CH], F32, tag="pj", name="pjt")
                        for kt in range(KT):
                            nc.tensor.matmul(
                                ps[:],
                                wtiles[wname][kt][:, d2 * P:(d2 + 1) * P],
                                xt[kt][:],
                                start=(kt == 0), stop=(kt == KT - 1),
                            )
                        csl = slice(c * CH, (c + 1) * CH)
                        nc.vector.tensor_scalar_mul(
                            dst65[2 * d2][0:DEP, csl], ps[0:DEP, :], scl)
                        nc.vector.tensor_scalar_mul(
                            dst65[2 * d2 + 1][0:DEP, csl], ps[DEP:P, :], scl)
                        nc.scalar.mul(dstB[d2][:, csl], ps[:], scl)

            # v projection -> [S, 4*65] f32r tiles + column sums
            for c in range(S // CH):
                xt = []
                for kt in range(KT):
                    x_t = xin.tile([P, CH], F32, tag=f"x{kt}", name=f"xvt{kt}")
                    nc.sync.dma_start(
                        out=x_t[:],
                        in_=xvT[kt * P:(kt + 1) * P, c * CH:(c + 1) * CH])
                    xt.append(x_t)
                for jt in range(4 * c, 4 * c + 4):
                  ps = pjv.tile([P, HC], F32, tag="pjv", name="pjvt")
                  for kt in range(KT):
                    nc.tensor.matmul(
                        ps[:],
                        xt[kt][:, (jt - 4 * c) * P:(jt - 4 * c + 1) * P],
                        wtiles["wv"][kt][:],
                        start=(kt == 0), stop=(kt == KT - 1),
                    )
                  for h in range(4):
                    nc.vector.tensor_copy(
                        out=vS[jt][:, h * VW:h * VW + DEP],
                        in_=ps[:, h * DEP:(h + 1) * DEP])
                    nc.vector.tensor_copy(
                        out=vS[jt][:, h * VW + DEP:(h + 1) * VW],
                        in_=ones_col[:])
                  psc = pjcs.tile([1, SB], F32, tag="pjcs", name="pjcst")
                  nc.tensor.matmul(psc[:], ones_r[:], vS[jt][:],
                                   start=True, stop=True)
                  nc.scalar.copy(out=cs_flat[0:1, jt * SB:(jt + 1) * SB],
                                 in_=psc[:])

        # suffix sums over j-tiles (includes the ones-column positions,
        # which are later overwritten with cnt_t)
        nc.vector.memset(sufv[0:1, (NT - 1) * SB: NT * SB], 0.0)
        for t in range(NT - 2, -1, -1):
            nc.vector.tensor_add(
                sufv[0:1, t * SB:(t + 1) * SB],
                sufv[0:1, (t + 1) * SB:(t + 2) * SB],
                cs_flat[0:1, (t + 1) * SB:(t + 2) * SB],
            )
        for t in range(NT):
            cnt = float(S - P * (t + 1))
            for h in range(4):
                nc.vector.memset(
                    sufv[0:1, t * SB + h * VW + DEP: t * SB + (h + 1) * VW],
                    cnt)

        # ---------------- phase 2: attention ----------------
        with tc.tile_pool(name="stp", bufs=2, space="PSUM") as stp, \
             tc.tile_pool(name="scp", bufs=2, space="PSUM") as scp, \
             tc.tile_pool(name="trp", bufs=1, space="PSUM") as trp, \
             tc.tile_pool(name="otp", bufs=2, space="PSUM") as otp, \
             tc.tile_pool(name="ptrow", bufs=1) as ptp, \
             tc.tile_pool(name="small", bufs=2) as smp, \
             tc.tile_pool(name="hold", bufs=1) as hold, \
             tc.tile_pool(name="osb", bufs=1) as osbp:

            for ic in range(NICH):
                tlist = list(range(4 * ic, 4 * ic + 4))
                outsb = {t: osbp.tile([P, HC], F32, tag=f"osb{t % 4}",
                                      name=f"osbt{t % 4}")
                         for t in tlist}
                for h in range(4):
                    pair, hh = divmod(h, 2)
                    pslo, pshi = hh * DEP, (hh + 1) * DEP
                    e8rows = {}
                    bcs = {}
                    # --- stats pass: m~ per i-tile, fold into q65 row 64 ---
                    for t in tlist:
                        L = P * (t + 1)
                        nch = (L + CH - 1) // CH
                        dlo = t * P - (nch - 1) * CH
                        rmp = smp.tile([P, nch], F32, tag="rmp", name="rmp")
                        for c in range(nch):
                            cl = min(CH, L - c * CH)
                            st = stp.tile([P, CH], F32, tag="st", name="stt")
                            nc.tensor.matmul(
                                st[:, :cl],
                                qB[pair][pslo:pshi, t * P:(t + 1) * P],
                                kB[pair][pslo:pshi, c * CH:c * CH + cl],
                                start=True, stop=True,
                            )
                            if c == nch - 1:
                                nc.vector.copy_predicated(
                                    st[:, dlo:dlo + P], umask[:], neg8[:])
                            nc.vector.reduce_max(
                                rmp[:, c:c + 1], st[:, :cl],
                                axis=mybir.AxisListType.X)
                        mm = smp.tile([P, 1], F32, tag="mm", name="mmt")
                        nc.vector.tensor_reduce(
                            mm[:], rmp[:], axis=mybir.AxisListType.X,
                            op=mybir.AluOpType.max)
                        nc.vector.tensor_scalar_max(mm[:], mm[:], -8.0)
                        # m~ row -> q65[h][64, t*P:(t+1)*P] via transpose + DMA
                        psm = trp.tile([1, P], F32, tag="tr", name="trm")
                        nc.tensor.transpose(psm[:], mm[:], ident[:])
                        mrow = smp.tile([1, P], F32, tag="mrow", name="mrowt")
                        nc.scalar.copy(out=mrow[:], in_=psm[:])
                        nc.sync.dma_start(
                            out=q65[h][DEP:VW, t * P:(t + 1) * P], in_=mrow[:])
                        # e8m = exp(-8 - m~) as a row (rank-1 rhs)
                        e8 = smp.tile([P, 1], F32, tag="e8m", name="e8mt")
                        nc.scalar.activation(
                            e8[:], mm[:], mybir.ActivationFunctionType.Exp,
                            bias=neg8_col[:], scale=-1.0)
                        pse = trp.tile([1, P], F32, tag="tr", name="tre")
                        nc.tensor.transpose(pse[:], e8[:], ident[:])
                        e8r = hold.tile([1, P], F32, tag=f"e8r{t % 4}",
                                        name=f"e8r{t % 4}")
                        nc.vector.tensor_copy(out=e8r[:], in_=pse[:])
                        e8rows[t] = e8r
                        # bc tile = broadcast of (-8 - m~_i): masked scores
                        nm8 = smp.tile([1, P], F32, tag="nm8", name="nm8t")
                        nc.vector.tensor_scalar_mul(nm8[:], mrow[:], -1.0)
                        nc.vector.tensor_scalar_add(nm8[:], nm8[:], -8.0)
                        bc = hold.tile([P, P], F32, tag=f"bc{t % 4}",
                                       name=f"bct{t % 4}")
                        nc.gpsimd.partition_broadcast(bc[:], nm8[:])
                        bcs[t] = bc
                    # --- transposed scores + exp -> f32r p tiles ---
                    ptrow = {}
                    for jb in range(4 * ic + 4):
                        i0 = max(0, jb - 4 * ic) * P
                        sc = scp.tile([P, CH], F32, tag="sc", name="sct")
                        nc.tensor.matmul(
                            sc[:, i0:CH],
                            k65[h][:, jb * P:(jb + 1) * P],
                            q65[h][:, ic * CH + i0:(ic + 1) * CH],
                            start=True, stop=True,
                        )
                        if jb >= 4 * ic:
                            # diagonal block: masked scores = -8 - m~_i so
                            # exp gives exp(-8 - m~_i)
                            nc.vector.copy_predicated(
                                sc[:, i0:i0 + P], lmask[:], bcs[jb][:])
                        pt = ptp.tile([P, CH], F32R, tag=f"pt{jb}",
                                      name=f"pt{jb}")
                        nc.scalar.activation(
                            pt[:, i0:CH], sc[:, i0:CH],
                            mybir.ActivationFunctionType.Exp, bias=0.0)
                        ptrow[jb] = pt
                    # --- PV: outT[d,i] + Z row via ones column ---
                    oT = otp.tile([VW, CH], F32, tag="oT", name="oTt")
                    for jb in range(4 * ic + 4):
                        i0 = max(0, jb - 4 * ic) * P
                        nc.tensor.matmul(
                            oT[:, i0:CH],
                            vS[jb][:, h * VW:(h + 1) * VW],
                            ptrow[jb][:, i0:CH],
                            start=(jb == 0), stop=False,
                        )
                    for t in tlist:
                        io = (t - 4 * ic) * P
                        nc.tensor.matmul(
                            oT[:, io:io + P],
                            sufv[0:1, t * SB + h * VW:t * SB + (h + 1) * VW],
                            e8rows[t][:],
                            start=False, stop=(t == tlist[-1]),
                        )
                    # --- back-transpose + normalize ---
                    oTsb = smp.tile([VW, CH], F32, tag="oTsb", name="oTsbt")
                    nc.scalar.copy(out=oTsb[:], in_=oT[:])
                    for t in tlist:
                        io = (t - 4 * ic) * P
                        ptr2 = trp.tile([P, VW], F32, tag="tr2", name="trt")
                        nc.tensor.transpose(
                            ptr2[:], oTsb[:, io:io + P], ident[0:VW, 0:VW])
                        rc = smp.tile([P, 1], F32, tag="rc", name="rct")
                        nc.vector.reciprocal(rc[:], ptr2[:, DEP:VW])
                        nc.vector.tensor_scalar_mul(
                            outsb[t][:, h * DEP:(h + 1) * DEP],
                            ptr2[:, 0:DEP], rc[:])
                for t in tlist:
                    nc.sync.dma_start(
                        out=outd[t * P:(t + 1) * P, :], in_=outsb[t][:])


_PROGRAM_CACHE = {}


def _build():
    if "nc" in _PROGRAM_CACHE:
        return _PROGRAM_CACHE["nc"]
    nc = bacc.Bacc(trn_type="TRN2", target_bir_lowering=False, debug=False)
    with tile.TileContext(nc) as tc:
        _emit(nc, tc)
    nc.compile()
    _PROGRAM_CACHE["nc"] = nc
    return nc


def kernel(q, k, v, mask, wq, wk, wv, _collect_results=None):
    q = np.asarray(q, dtype=np.float32)
    k = np.asarray(k, dtype=np.float32)
    v = np.asarray(v, dtype=np.float32)
    wq = np.asarray(wq, dtype=np.float32)
    wk = np.asarray(wk, dtype=np.float32)
    wv = np.asarray(wv, dtype=np.float32)
    mask = np.asarray(mask)

    # the device program hardcodes the causal structure
    mrow = np.broadcast_to(mask, (1, 1, S, S)).reshape(S, S)
    tril = np.tril(np.ones((S, S), dtype=bool))
    if not np.array_equal(mrow.astype(bool), tril):
        raise NotImplementedError("kernel compiled for causal (tril) mask only")

    nc = _build()

    in_maps = []
    for core in range(8):
        b, m = divmod(core, 4)
        cols = slice(HC * m, HC * (m + 1))
        in_maps.append({
            "xqT": np.ascontiguousarray(q[b].T),
            "xkT": np.ascontiguousarray(k[b].T),
            "xvT": np.ascontiguousarray(v[b].T),
            "wq": np.ascontiguousarray(wq[:, cols]),
            "wk": np.ascontiguousarray(wk[:, cols]),
            "wv": np.ascontiguousarray(wv[:, cols]),
        })

    trace = _collect_results is not None
    res = run_bass_kernel_spmd(nc, in_maps, core_ids=list(range(8)), trace=trace)
    if _collect_results is not None:
        _collect_results.append(res)

    out = np.empty((2, S, D), dtype=np.float32)
    for core in range(8):
        b, m = divmod(core, 4)
        out[b, :, HC * m: HC * (m + 1)] = res.results[core]["out"]
    return out
